# revision 16
# baseline (speedup 1.0000x reference)
"""Causal self-attention (B=4, T=2048, C=2048, H=16, hd=128) on 8 trn2 cores.

Sharding: core = b*2 + half. Each core handles batch b and 8 heads
(half*8 .. half*8+7): tensor-parallel over heads within a batch, data
parallel over batch. Each core computes a partial out-projection
(contribution of its 8 heads); host sums the two partials per batch.

Device kernel (identical program on all cores, different data):
  P1: qT/kT = W_{q,k}^T-chunks @ xT  (RoPE applied, even/odd dims
      pre-permuted into partition halves via host weight permutation),
      v = x @ Wv^T (natural layout). All staged through DRAM scratch.
  P2: per head: transposed scores sT[k,q] = kT^T qT, causal mask,
      exp on ACT, attV + ones-row column sums on PE, normalize.
  P3: partial out-proj from SBUF-resident yT.

All matmuls run as float32r (full-rate fp32 on the PE).
"""

import numpy as np

import concourse.bass as bass
import concourse.tile as tile
from concourse import bacc, bass2jax, mybir

F32 = mybir.dt.float32
F32R = mybir.dt.float32r

B = 4
T = 2048
C = 2048
HD = 128
HL = 8          # local heads per core
NCC = 16        # contraction chunks of 128 over C
NTB = 16        # t blocks of 128
NQS = 4         # q strips of 512
SW = 512
N_CORES = 8
NEG = -1.0e30


def _r(ap):
    return ap


def build_program():
    nc = bacc.Bacc(None, target_bir_lowering=False)

    xT = nc.declare_dram_parameter("xT", [NCC, 128, T], F32R, isOutput=False)
    wq = nc.declare_dram_parameter("wq", [HL, 128, C], F32R, isOutput=False)
    wk = nc.declare_dram_parameter("wk", [HL, 128, C], F32R, isOutput=False)
    wv = nc.declare_dram_parameter("wv", [2, 128, 16 * SW], F32R, isOutput=False)
    wp = nc.declare_dram_parameter("wp", [HL, 128, C], F32R, isOutput=False)
    cs = nc.declare_dram_parameter("cs", [128, T], F32, isOutput=False)
    ss = nc.declare_dram_parameter("ss", [128, T], F32, isOutput=False)
    ones_in = nc.declare_dram_parameter("ones_in", [128, 128], F32R, isOutput=False)
    out = nc.declare_dram_parameter("out", [T, C], F32, isOutput=True)

    qs = nc.dram_tensor("qs", [HL, 128, T], F32R)
    ks = nc.dram_tensor("ks", [HL, 128, T], F32R)
    vs = nc.dram_tensor("vs", [NTB, 128, HL * 128], F32R)

    with tile.TileContext(nc) as tc:
        with tc.tile_pool(name="const", bufs=1) as cpool:
            ones_col = cpool.tile([128, 1], F32R, name="ones_col", tag="oc")
            nc.sync.dma_start(out=ones_col[:], in_=ones_in[:, 0:1])
            ones_row = cpool.tile([1, 128], F32R, name="ones_row", tag="orow")
            nc.sync.dma_start(out=ones_row[:], in_=ones_in[0:1, :])
            # maskT[k, q] = 0 where k <= q else NEG  (k = partition)
            maskT = cpool.tile([128, 128], F32, name="maskT", tag="mask")
            nc.vector.memset(maskT[:], 0.0)
            # keep 0.0 where q - k >= 0 (k = partition, q = free), else NEG
            nc.gpsimd.affine_select(
                out=maskT[:],
                in_=maskT[:],
                compare_op=mybir.AluOpType.is_ge,
                fill=NEG,
                base=0,
                pattern=[[1, 128]],
                channel_multiplier=-1,
            )
            cs_sb = cpool.tile([128, T], F32, name="cs_sb", tag="cs")
            nc.sync.dma_start(out=cs_sb[:], in_=cs[:])
            ss_sb = cpool.tile([128, T], F32, name="ss_sb", tag="ss")
            nc.sync.dma_start(out=ss_sb[:], in_=ss[:])

            # ---------------- P1: projections ----------------
            with tc.tile_pool(name="xin", bufs=1) as xpool:
                xsb = []
                for cc in range(NCC):
                    xt = xpool.tile([128, T], F32R, name=f"xsb{cc}", tag=f"x{cc}")
                    nc.sync.dma_start(out=xt[:], in_=xT[cc])
                    xsb.append(xt)

                # q/k projections + rope
                with (
                    tc.tile_pool(name="wqk", bufs=2) as wpool,
                    tc.tile_pool(name="qkps", bufs=8, space="PSUM") as qkpool,
                    tc.tile_pool(name="rope", bufs=1) as rpool,
                ):
                    for w_in, dst in ((wq, qs), (wk, ks)):
                        for h in range(HL):
                            wsb = wpool.tile(
                                [128, C], F32R, name=f"wsb_{h}", tag="wqk"
                            )
                            nc.sync.dma_start(out=wsb[:], in_=w_in[h])
                            pss = [
                                qkpool.tile(
                                    [128, SW], F32, name=f"pqk{h}_{s}", tag="qkps"
                                )
                                for s in range(NQS)
                            ]
                            for cc in range(NCC):
                                for s in range(NQS):
                                    nc.tensor.matmul(
                                        pss[s][:],
                                        _r(wsb[:, cc * 128 : (cc + 1) * 128]),
                                        _r(xsb[cc][:, s * SW : (s + 1) * SW]),
                                        start=(cc == 0),
                                        stop=(cc == NCC - 1),
                                    )
                            swap_mask = list(range(16, 32)) + list(range(16))
                            for s in range(NQS):
                                qt = rpool.tile(
                                    [128, SW], F32, name="qt", tag="qt", bufs=3
                                )
                                nc.vector.tensor_copy(qt[:], pss[s][:])
                                csl = cs_sb[:, s * SW : (s + 1) * SW]
                                snl = ss_sb[:, s * SW : (s + 1) * SW]
                                t1 = rpool.tile(
                                    [128, SW], F32, name="t1", tag="t1", bufs=2
                                )
                                t2 = rpool.tile(
                                    [128, SW], F32, name="t2", tag="t2", bufs=2
                                )
                                qsw = rpool.tile(
                                    [128, SW], F32, name="qsw", tag="qsw", bufs=2
                                )
                                nc.vector.tensor_mul(t1[:], qt[:], csl)
                                nc.vector.stream_shuffle(qsw[:], qt[:], swap_mask)
                                nc.vector.tensor_mul(t2[:], qsw[:], snl)
                                qf = rpool.tile(
                                    [128, SW], F32R, name="qf", tag="qf", bufs=3
                                )
                                nc.vector.tensor_add(qf[:], t1[:], t2[:])
                                nc.sync.dma_start(
                                    out=dst[h][:, s * SW : (s + 1) * SW],
                                    in_=qf[:],
                                )

                # v projection (natural layout), 4 heads at a time
                with (
                    tc.tile_pool(name="wvp", bufs=1) as wvpool,
                    tc.tile_pool(name="vps", bufs=2, space="PSUM") as vpspool,
                    tc.tile_pool(name="vcp", bufs=3) as vcpool,
                ):
                    for qd in range(2):
                        wvsb = wvpool.tile(
                            [128, 16 * SW], F32R, name=f"wvsb{qd}", tag="wv"
                        )
                        nc.sync.dma_start(out=wvsb[:], in_=wv[qd])
                        for tb in range(NTB):
                            pv = vpspool.tile(
                                [128, SW], F32, name="pv", tag="vps"
                            )
                            for cc in range(NCC):
                                nc.tensor.matmul(
                                    pv[:],
                                    _r(xsb[cc][:, tb * 128 : (tb + 1) * 128]),
                                    _r(wvsb[:, cc * SW : (cc + 1) * SW]),
                                    start=(cc == 0),
                                    stop=(cc == NCC - 1),
                                )
                            vcp = vcpool.tile([128, SW], F32R, name="vcp", tag="vcp")
                            nc.vector.tensor_copy(vcp[:], pv[:])
                            nc.sync.dma_start(
                                out=vs[tb][:, qd * SW : (qd + 1) * SW],
                                in_=vcp[:],
                            )

            # ---------------- P2: attention ----------------
            with tc.tile_pool(name="ybig", bufs=1) as ypool:
                ysb = [
                    ypool.tile([128, T], F32R, name=f"ysb{h}", tag=f"y{h}")
                    for h in range(HL)
                ]
                self_attention_and_proj(
                    nc, tc, ysb, qs, ks, vs, wp, out,
                    ones_col, ones_row, maskT,
                )

    nc.compile()
    return nc


def self_attention_and_proj(nc, tc, ysb, qs, ks, vs, wp, out, ones_col, ones_row, maskT):
            with (
                tc.tile_pool(name="qkh", bufs=2) as qkhpool,
                tc.tile_pool(name="stp", bufs=2, space="PSUM") as stpool,
                tc.tile_pool(name="op", bufs=2, space="PSUM") as oppool,
                tc.tile_pool(name="sump", bufs=2, space="PSUM") as sumpool,
                tc.tile_pool(name="bp", bufs=2, space="PSUM") as bppool,
                tc.tile_pool(name="esb", bufs=3) as epool,
                tc.tile_pool(name="nrm", bufs=2) as npool,
            ):
                for h in range(HL):
                    qsb_h = qkhpool.tile([128, T], F32R, name=f"qh{h}", tag="qh")
                    nc.sync.dma_start(out=qsb_h[:], in_=qs[h])
                    ksb_h = qkhpool.tile([128, T], F32R, name=f"kh{h}", tag="kh")
                    nc.sync.dma_start(out=ksb_h[:], in_=ks[h])
                    vsb_h = qkhpool.tile(
                        [128, NTB, 128], F32R, name=f"vh{h}", tag="vh"
                    )
                    for tb in range(NTB):
                        nc.sync.dma_start(
                            out=vsb_h[:, tb, :],
                            in_=vs[tb][:, h * 128 : (h + 1) * 128],
                        )

                    for g in range(NQS):
                        po = oppool.tile([128, SW], F32, name="po", tag="po")
                        psum = sumpool.tile([1, SW], F32, name="psum", tag="ps")
                        nkb = 4 * g + 4
                        for kb in range(nkb):
                            off = 128 * max(0, kb - 4 * g)
                            pst = stpool.tile([128, SW], F32, name="pst", tag="pst")
                            nc.tensor.matmul(
                                pst[:, off:SW],
                                _r(ksb_h[:, kb * 128 : (kb + 1) * 128]),
                                _r(qsb_h[:, g * SW + off : (g + 1) * SW]),
                            )
                            if off:
                                nc.vector.memset(pst[:, 0:off], NEG)
                            if kb >= 4 * g:
                                nc.vector.tensor_add(
                                    pst[:, off : off + 128],
                                    pst[:, off : off + 128],
                                    maskT[:],
                                )
                            esb = epool.tile([128, SW], F32R, name="esb", tag="esb")
                            nc.scalar.activation(
                                esb[:],
                                pst[:],
                                mybir.ActivationFunctionType.Exp,
                            )
                            nc.tensor.matmul(
                                po[:],
                                _r(vsb_h[:, kb, :]),
                                _r(esb[:]),
                                start=(kb == 0),
                                stop=(kb == nkb - 1),
                            )
                            nc.tensor.matmul(
                                psum[:],
                                _r(ones_col[:]),
                                _r(esb[:]),
                                start=(kb == 0),
                                stop=(kb == nkb - 1),
                            )
                        recip = npool.tile([1, SW], F32, name="recip", tag="recip")
                        nc.vector.reciprocal(recip[:], psum[:])
                        recip_r = npool.tile([1, SW], F32R, name="recip_r", tag="recr")
                        nc.vector.tensor_copy(recip_r[:], recip[:])
                        pb = bppool.tile([128, SW], F32, name="pb", tag="pb")
                        nc.tensor.matmul(pb[:], _r(ones_row[:]), _r(recip_r[:]))
                        bsb = npool.tile([128, SW], F32, name="bsb", tag="bsb")
                        nc.vector.tensor_copy(bsb[:], pb[:])
                        nc.vector.tensor_mul(
                            ysb[h][:, g * SW : (g + 1) * SW], po[:], bsb[:]
                        )

            # ---------------- P3: out projection ----------------
            with (
                tc.tile_pool(name="wpp", bufs=1) as wppool,
                tc.tile_pool(name="fps", bufs=4, space="PSUM") as fpool,
                tc.tile_pool(name="osb", bufs=3) as ospool,
            ):
                wpsb = []
                for cb in range(HL):
                    wt = wppool.tile([128, C], F32R, name=f"wp{cb}", tag=f"wp{cb}")
                    nc.sync.dma_start(out=wt[:], in_=wp[cb])
                    wpsb.append(wt)
                for tb in range(NTB):
                    for csi in range(4):
                        pf = fpool.tile([128, SW], F32, name="pf", tag="pf")
                        for cb in range(HL):
                            nc.tensor.matmul(
                                pf[:],
                                _r(ysb[cb][:, tb * 128 : (tb + 1) * 128]),
                                _r(wpsb[cb][:, csi * SW : (csi + 1) * SW]),
                                start=(cb == 0),
                                stop=(cb == HL - 1),
                            )
                        osb = ospool.tile([128, SW], F32, name="osb", tag="osb")
                        nc.vector.tensor_copy(osb[:], pf[:])
                        nc.sync.dma_start(
                            out=out[
                                tb * 128 : (tb + 1) * 128,
                                csi * SW : (csi + 1) * SW,
                            ],
                            in_=osb[:],
                        )


# Per-head permutation of the 128 head dims: quadrant qd (32 partitions)
# holds rope pairs p = qd*16..qd*16+15 — even dims (2p) in slots 0..15,
# odd dims (2p+1) in slots 16..31. The rope partner swap is then a
# within-quadrant stream_shuffle by +-16.
_PERM = np.concatenate(
    [
        np.concatenate([2 * (qd * 16 + np.arange(16)) + r for r in (0, 1)])
        for qd in range(4)
    ]
)
# pair index held by each partition slot
_PAIR_OF_SLOT = np.concatenate(
    [np.tile(qd * 16 + np.arange(16), 2) for qd in range(4)]
)
# +1 on odd slots, -1 on even slots (sign of the sin term)
_SIN_SIGN = np.concatenate([np.repeat([-1.0, 1.0], 16) for _ in range(4)])


def prepare_core_inputs(x, Wq, Wk, Wv, Wp):
    """Returns list of 8 input dicts, core = b*2 + half."""
    scale = 1.0 / np.sqrt(HD)

    inv_freq = (1.0 / (10000.0 ** (np.arange(0, HD, 2) / HD))).astype(np.float64)
    freqs = np.outer(inv_freq[_PAIR_OF_SLOT], np.arange(T, dtype=np.float64))
    cs = np.cos(freqs).astype(np.float32)
    ss = (np.sin(freqs) * _SIN_SIGN[:, None]).astype(np.float32)

    halves = []
    for half in range(2):
        r0 = half * HL * HD  # first global row of this half's heads
        wq_in = np.empty((HL, 128, C), np.float32)
        wk_in = np.empty((HL, 128, C), np.float32)
        for h in range(HL):
            for arr, W, sc in ((wq_in, Wq, scale), (wk_in, Wk, 1.0)):
                Wh = W[r0 + h * HD : r0 + (h + 1) * HD][_PERM] * sc  # [128 d, C]
                # arr[h, p, cc*128+d] = Wh[d, cc*128+p]
                arr[h] = np.ascontiguousarray(
                    Wh.reshape(128, NCC, 128).transpose(2, 1, 0).reshape(128, C)
                )
        Wv_half = Wv[r0 : r0 + HL * HD]  # [1024, C]
        wv_in = np.empty((2, 128, 16 * SW), np.float32)
        for qd in range(2):
            Wv4 = Wv_half[qd * SW : (qd + 1) * SW]  # [512 d4, C]
            wv_in[qd] = (
                Wv4.reshape(SW, NCC, 128).transpose(2, 1, 0).reshape(128, NCC * SW)
            )
        wp_in = np.ascontiguousarray(
            Wp.T[r0 : r0 + HL * HD].reshape(HL, 128, C)
        )
        halves.append((wq_in, wk_in, wv_in, wp_in))

    in_maps = []
    for b in range(B):
        xTb = np.ascontiguousarray(x[b].T).reshape(NCC, 128, T)
        for half in range(2):
            wq_in, wk_in, wv_in, wp_in = halves[half]
            in_maps.append(
                {
                    "xT": xTb,
                    "wq": wq_in,
                    "wk": wk_in,
                    "wv": wv_in,
                    "wp": wp_in,
                    "cs": cs,
                    "ss": ss,
                    "ones_in": np.ones((128, 128), np.float32),
                }
            )
    return in_maps


_RUNNER_CACHE = None


def _make_runner():
    """Compile the Bass program once and return a callable
    (in_maps -> list of per-core output dicts) that reuses the jitted
    executable across calls. Mirrors bass2jax.run_bass_via_pjrt's
    multi-core branch."""
    import jax
    from jax.experimental.shard_map import shard_map
    from jax.sharding import Mesh, PartitionSpec

    nc = build_program()
    bass2jax.install_neuronx_cc_hook()

    partition_name = nc.partition_id_tensor.name if nc.partition_id_tensor else None
    in_names, out_names, out_avals, zero_shapes = [], [], [], []
    for alloc in nc.m.functions[0].allocations:
        if not isinstance(alloc, mybir.MemoryLocationSet):
            continue
        name = alloc.memorylocations[0].name
        if alloc.kind == "ExternalInput":
            if name != partition_name:
                in_names.append(name)
        elif alloc.kind == "ExternalOutput":
            shape = tuple(alloc.tensor_shape)
            dtype = mybir.dt.np(alloc.dtype)
            out_names.append(name)
            out_avals.append(jax.core.ShapedArray(shape, dtype))
            zero_shapes.append((shape, dtype))
    n_params = len(in_names)
    n_outs = len(out_avals)
    all_in_names = list(in_names) + list(out_names)
    if partition_name is not None:
        all_in_names.append(partition_name)
    donate = tuple(range(n_params, n_params + n_outs))

    def _body(*args):
        operands = list(args)
        if partition_name is not None:
            operands.append(bass2jax.partition_id_tensor())
        outs = bass2jax._bass_exec_p.bind(
            *operands,
            out_avals=tuple(out_avals),
            in_names=tuple(all_in_names),
            out_names=tuple(out_names),
            lowering_input_output_aliases=(),
            sim_require_finite=True,
            sim_require_nnan=True,
            nc=nc,
        )
        return tuple(outs)

    devices = jax.devices()[:N_CORES]
    mesh = Mesh(np.asarray(devices), ("core",))
    in_specs = (PartitionSpec("core"),) * (n_params + n_outs)
    out_specs = (PartitionSpec("core"),) * n_outs
    sharded = jax.jit(
        shard_map(
            _body, mesh=mesh, in_specs=in_specs, out_specs=out_specs, check_rep=False
        ),
        donate_argnums=donate,
        keep_unused=True,
    )

    def run(in_maps):
        concat_in = [
            np.concatenate([np.asarray(m[name]) for m in in_maps], axis=0)
            for name in in_names
        ]
        concat_zeros = [
            np.zeros((N_CORES * s[0], *s[1:]), d) for (s, d) in zero_shapes
        ]
        out_arrs = sharded(*concat_in, *concat_zeros)
        return [
            {
                name: np.asarray(out_arrs[i]).reshape(
                    N_CORES, *out_avals[i].shape
                )[c]
                for i, name in enumerate(out_names)
            }
            for c in range(N_CORES)
        ]

    return run


def get_runner():
    global _RUNNER_CACHE
    if _RUNNER_CACHE is None:
        _RUNNER_CACHE = _make_runner()
    return _RUNNER_CACHE


def kernel(x, Wq, Wk, Wv, Wp):
    run = get_runner()
    in_maps = prepare_core_inputs(
        np.asarray(x), np.asarray(Wq), np.asarray(Wk), np.asarray(Wv), np.asarray(Wp)
    )
    res = run(in_maps)
    out = np.empty((B, T, C), np.float32)
    for b in range(B):
        np.add(res[2 * b]["out"], res[2 * b + 1]["out"], out=out[b])
    return out


# revision 18
# speedup vs baseline: 8906.1146x; 8906.1146x over previous
"""Causal self-attention (B=4, T=2048, C=2048, H=16, hd=128) on 8 trn2 cores.

Sharding: core = b*2 + half. Each core handles batch b and 8 heads
(half*8 .. half*8+7): tensor-parallel over heads within a batch, data
parallel over batch. Each core computes a partial out-projection
(contribution of its 8 heads); host sums the two partials per batch.

Device kernel (identical program on all cores, different data):
  P1: qT/kT = W_{q,k}^T-chunks @ xT  (RoPE applied, even/odd dims
      pre-permuted into partition halves via host weight permutation),
      v = x @ Wv^T (natural layout). All staged through DRAM scratch.
  P2: per head: transposed scores sT[k,q] = kT^T qT, causal mask,
      exp on ACT, attV + ones-row column sums on PE, normalize.
  P3: partial out-proj from SBUF-resident yT.

All matmuls run as float32r (full-rate fp32 on the PE).
"""

import numpy as np

import concourse.bass as bass
import concourse.tile as tile
from concourse import bacc, bass2jax, mybir

F32 = mybir.dt.float32
F32R = mybir.dt.float32r

B = 4
T = 2048
C = 2048
HD = 128
HL = 8          # local heads per core
NCC = 16        # contraction chunks of 128 over C
NTB = 16        # t blocks of 128
NQS = 4         # q strips of 512
SW = 512
N_CORES = 8
NEG = -1.0e30


def _r(ap):
    return ap


def build_program():
    nc = bacc.Bacc(None, target_bir_lowering=False)

    xT = nc.declare_dram_parameter("xT", [NCC, 128, T], F32R, isOutput=False)
    wq = nc.declare_dram_parameter("wq", [HL, 128, C], F32R, isOutput=False)
    wk = nc.declare_dram_parameter("wk", [HL, 128, C], F32R, isOutput=False)
    wv = nc.declare_dram_parameter("wv", [2, 128, 16 * SW], F32R, isOutput=False)
    wp = nc.declare_dram_parameter("wp", [HL, 128, C], F32R, isOutput=False)
    cs = nc.declare_dram_parameter("cs", [128, T], F32, isOutput=False)
    ss = nc.declare_dram_parameter("ss", [128, T], F32, isOutput=False)
    ones_in = nc.declare_dram_parameter("ones_in", [128, 128], F32R, isOutput=False)
    out = nc.declare_dram_parameter("out", [T, C], F32, isOutput=True)

    qs = nc.dram_tensor("qs", [HL, 128, T], F32R)
    ks = nc.dram_tensor("ks", [HL, 128, T], F32R)
    vs = nc.dram_tensor("vs", [NTB, 128, HL * 128], F32R)

    with tile.TileContext(nc) as tc:
        with tc.tile_pool(name="const", bufs=1) as cpool:
            ones_col = cpool.tile([128, 1], F32R, name="ones_col", tag="oc")
            nc.sync.dma_start(out=ones_col[:], in_=ones_in[:, 0:1])
            ones_row = cpool.tile([1, 128], F32R, name="ones_row", tag="orow")
            nc.sync.dma_start(out=ones_row[:], in_=ones_in[0:1, :])
            # maskT[k, q] = 0 where k <= q else NEG  (k = partition)
            maskT = cpool.tile([128, 128], F32, name="maskT", tag="mask")
            nc.vector.memset(maskT[:], 0.0)
            # keep 0.0 where q - k >= 0 (k = partition, q = free), else NEG
            nc.gpsimd.affine_select(
                out=maskT[:],
                in_=maskT[:],
                compare_op=mybir.AluOpType.is_ge,
                fill=NEG,
                base=0,
                pattern=[[1, 128]],
                channel_multiplier=-1,
            )
            cs_sb = cpool.tile([128, T], F32, name="cs_sb", tag="cs")
            nc.sync.dma_start(out=cs_sb[:], in_=cs[:])
            ss_sb = cpool.tile([128, T], F32, name="ss_sb", tag="ss")
            nc.sync.dma_start(out=ss_sb[:], in_=ss[:])

            # ---------------- P1: projections ----------------
            with tc.tile_pool(name="xin", bufs=1) as xpool:
                xsb = []
                for cc in range(NCC):
                    xt = xpool.tile([128, T], F32R, name=f"xsb{cc}", tag=f"x{cc}")
                    nc.sync.dma_start(out=xt[:], in_=xT[cc])
                    xsb.append(xt)

                # q/k projections + rope
                with (
                    tc.tile_pool(name="wqk", bufs=2) as wpool,
                    tc.tile_pool(name="qkps", bufs=8, space="PSUM") as qkpool,
                    tc.tile_pool(name="rope", bufs=1) as rpool,
                ):
                    for w_in, dst in ((wq, qs), (wk, ks)):
                        for h in range(HL):
                            wsb = wpool.tile(
                                [128, C], F32R, name=f"wsb_{h}", tag="wqk"
                            )
                            nc.sync.dma_start(out=wsb[:], in_=w_in[h])
                            pss = [
                                qkpool.tile(
                                    [128, SW], F32, name=f"pqk{h}_{s}", tag="qkps"
                                )
                                for s in range(NQS)
                            ]
                            for cc in range(NCC):
                                for s in range(NQS):
                                    nc.tensor.matmul(
                                        pss[s][:],
                                        _r(wsb[:, cc * 128 : (cc + 1) * 128]),
                                        _r(xsb[cc][:, s * SW : (s + 1) * SW]),
                                        start=(cc == 0),
                                        stop=(cc == NCC - 1),
                                    )
                            swap_mask = list(range(16, 32)) + list(range(16))
                            for s in range(NQS):
                                qt = rpool.tile(
                                    [128, SW], F32, name="qt", tag="qt", bufs=3
                                )
                                nc.vector.tensor_copy(qt[:], pss[s][:])
                                csl = cs_sb[:, s * SW : (s + 1) * SW]
                                snl = ss_sb[:, s * SW : (s + 1) * SW]
                                t1 = rpool.tile(
                                    [128, SW], F32, name="t1", tag="t1", bufs=2
                                )
                                t2 = rpool.tile(
                                    [128, SW], F32, name="t2", tag="t2", bufs=2
                                )
                                qsw = rpool.tile(
                                    [128, SW], F32, name="qsw", tag="qsw", bufs=2
                                )
                                nc.vector.tensor_mul(t1[:], qt[:], csl)
                                nc.vector.stream_shuffle(qsw[:], qt[:], swap_mask)
                                nc.vector.tensor_mul(t2[:], qsw[:], snl)
                                qf = rpool.tile(
                                    [128, SW], F32R, name="qf", tag="qf", bufs=3
                                )
                                nc.vector.tensor_add(qf[:], t1[:], t2[:])
                                nc.sync.dma_start(
                                    out=dst[h][:, s * SW : (s + 1) * SW],
                                    in_=qf[:],
                                )

                # v projection (natural layout), 4 heads at a time
                with (
                    tc.tile_pool(name="wvp", bufs=1) as wvpool,
                    tc.tile_pool(name="vps", bufs=2, space="PSUM") as vpspool,
                    tc.tile_pool(name="vcp", bufs=3) as vcpool,
                ):
                    for qd in range(2):
                        wvsb = wvpool.tile(
                            [128, 16 * SW], F32R, name=f"wvsb{qd}", tag="wv"
                        )
                        nc.sync.dma_start(out=wvsb[:], in_=wv[qd])
                        for tb in range(NTB):
                            pv = vpspool.tile(
                                [128, SW], F32, name="pv", tag="vps"
                            )
                            for cc in range(NCC):
                                nc.tensor.matmul(
                                    pv[:],
                                    _r(xsb[cc][:, tb * 128 : (tb + 1) * 128]),
                                    _r(wvsb[:, cc * SW : (cc + 1) * SW]),
                                    start=(cc == 0),
                                    stop=(cc == NCC - 1),
                                )
                            vcp = vcpool.tile([128, SW], F32R, name="vcp", tag="vcp")
                            nc.vector.tensor_copy(vcp[:], pv[:])
                            nc.sync.dma_start(
                                out=vs[tb][:, qd * SW : (qd + 1) * SW],
                                in_=vcp[:],
                            )

            # ---------------- P2: attention ----------------
            with tc.tile_pool(name="ybig", bufs=1) as ypool:
                ysb = [
                    ypool.tile([128, T], F32R, name=f"ysb{h}", tag=f"y{h}")
                    for h in range(HL)
                ]
                self_attention_and_proj(
                    nc, tc, ysb, qs, ks, vs, wp, out,
                    ones_col, ones_row, maskT,
                )

    nc.compile()
    return nc


def self_attention_and_proj(nc, tc, ysb, qs, ks, vs, wp, out, ones_col, ones_row, maskT):
            with (
                tc.tile_pool(name="qkh", bufs=2) as qkhpool,
                tc.tile_pool(name="stp", bufs=2, space="PSUM") as stpool,
                tc.tile_pool(name="op", bufs=2, space="PSUM") as oppool,
                tc.tile_pool(name="sump", bufs=2, space="PSUM") as sumpool,
                tc.tile_pool(name="bp", bufs=2, space="PSUM") as bppool,
                tc.tile_pool(name="esb", bufs=3) as epool,
                tc.tile_pool(name="nrm", bufs=2) as npool,
            ):
                for h in range(HL):
                    qsb_h = qkhpool.tile([128, T], F32R, name=f"qh{h}", tag="qh")
                    nc.sync.dma_start(out=qsb_h[:], in_=qs[h])
                    ksb_h = qkhpool.tile([128, T], F32R, name=f"kh{h}", tag="kh")
                    nc.sync.dma_start(out=ksb_h[:], in_=ks[h])
                    vsb_h = qkhpool.tile(
                        [128, NTB, 128], F32R, name=f"vh{h}", tag="vh"
                    )
                    for tb in range(NTB):
                        nc.sync.dma_start(
                            out=vsb_h[:, tb, :],
                            in_=vs[tb][:, h * 128 : (h + 1) * 128],
                        )

                    for g in range(NQS):
                        po = oppool.tile([128, SW], F32, name="po", tag="po")
                        psum = sumpool.tile([1, SW], F32, name="psum", tag="ps")
                        nkb = 4 * g + 4
                        for kb in range(nkb):
                            off = 128 * max(0, kb - 4 * g)
                            pst = stpool.tile([128, SW], F32, name="pst", tag="pst")
                            nc.tensor.matmul(
                                pst[:, off:SW],
                                _r(ksb_h[:, kb * 128 : (kb + 1) * 128]),
                                _r(qsb_h[:, g * SW + off : (g + 1) * SW]),
                            )
                            if off:
                                nc.vector.memset(pst[:, 0:off], NEG)
                            if kb >= 4 * g:
                                nc.vector.tensor_add(
                                    pst[:, off : off + 128],
                                    pst[:, off : off + 128],
                                    maskT[:],
                                )
                            esb = epool.tile([128, SW], F32R, name="esb", tag="esb")
                            nc.scalar.activation(
                                esb[:],
                                pst[:],
                                mybir.ActivationFunctionType.Exp,
                            )
                            nc.tensor.matmul(
                                po[:],
                                _r(vsb_h[:, kb, :]),
                                _r(esb[:]),
                                start=(kb == 0),
                                stop=(kb == nkb - 1),
                            )
                            nc.tensor.matmul(
                                psum[:],
                                _r(ones_col[:]),
                                _r(esb[:]),
                                start=(kb == 0),
                                stop=(kb == nkb - 1),
                            )
                        recip = npool.tile([1, SW], F32, name="recip", tag="recip")
                        nc.vector.reciprocal(recip[:], psum[:])
                        recip_r = npool.tile([1, SW], F32R, name="recip_r", tag="recr")
                        nc.vector.tensor_copy(recip_r[:], recip[:])
                        pb = bppool.tile([128, SW], F32, name="pb", tag="pb")
                        nc.tensor.matmul(pb[:], _r(ones_row[:]), _r(recip_r[:]))
                        bsb = npool.tile([128, SW], F32, name="bsb", tag="bsb")
                        nc.vector.tensor_copy(bsb[:], pb[:])
                        nc.vector.tensor_mul(
                            ysb[h][:, g * SW : (g + 1) * SW], po[:], bsb[:]
                        )

            # ---------------- P3: out projection ----------------
            with (
                tc.tile_pool(name="wpp", bufs=1) as wppool,
                tc.tile_pool(name="fps", bufs=4, space="PSUM") as fpool,
                tc.tile_pool(name="osb", bufs=3) as ospool,
            ):
                wpsb = []
                for cb in range(HL):
                    wt = wppool.tile([128, C], F32R, name=f"wp{cb}", tag=f"wp{cb}")
                    nc.sync.dma_start(out=wt[:], in_=wp[cb])
                    wpsb.append(wt)
                for tb in range(NTB):
                    for csi in range(4):
                        pf = fpool.tile([128, SW], F32, name="pf", tag="pf")
                        for cb in range(HL):
                            nc.tensor.matmul(
                                pf[:],
                                _r(ysb[cb][:, tb * 128 : (tb + 1) * 128]),
                                _r(wpsb[cb][:, csi * SW : (csi + 1) * SW]),
                                start=(cb == 0),
                                stop=(cb == HL - 1),
                            )
                        osb = ospool.tile([128, SW], F32, name="osb", tag="osb")
                        nc.vector.tensor_copy(osb[:], pf[:])
                        nc.sync.dma_start(
                            out=out[
                                tb * 128 : (tb + 1) * 128,
                                csi * SW : (csi + 1) * SW,
                            ],
                            in_=osb[:],
                        )


# Per-head permutation of the 128 head dims: quadrant qd (32 partitions)
# holds rope pairs p = qd*16..qd*16+15 — even dims (2p) in slots 0..15,
# odd dims (2p+1) in slots 16..31. The rope partner swap is then a
# within-quadrant stream_shuffle by +-16.
_PERM = np.concatenate(
    [
        np.concatenate([2 * (qd * 16 + np.arange(16)) + r for r in (0, 1)])
        for qd in range(4)
    ]
)
# pair index held by each partition slot
_PAIR_OF_SLOT = np.concatenate(
    [np.tile(qd * 16 + np.arange(16), 2) for qd in range(4)]
)
# +1 on odd slots, -1 on even slots (sign of the sin term)
_SIN_SIGN = np.concatenate([np.repeat([-1.0, 1.0], 16) for _ in range(4)])


def prepare_core_inputs(x, Wq, Wk, Wv, Wp):
    """Returns list of 8 input dicts, core = b*2 + half."""
    scale = 1.0 / np.sqrt(HD)

    inv_freq = (1.0 / (10000.0 ** (np.arange(0, HD, 2) / HD))).astype(np.float64)
    freqs = np.outer(inv_freq[_PAIR_OF_SLOT], np.arange(T, dtype=np.float64))
    cs = np.cos(freqs).astype(np.float32)
    ss = (np.sin(freqs) * _SIN_SIGN[:, None]).astype(np.float32)

    halves = []
    for half in range(2):
        r0 = half * HL * HD  # first global row of this half's heads
        wq_in = np.empty((HL, 128, C), np.float32)
        wk_in = np.empty((HL, 128, C), np.float32)
        for h in range(HL):
            for arr, W, sc in ((wq_in, Wq, scale), (wk_in, Wk, 1.0)):
                Wh = W[r0 + h * HD : r0 + (h + 1) * HD][_PERM] * sc  # [128 d, C]
                # arr[h, p, cc*128+d] = Wh[d, cc*128+p]
                arr[h] = np.ascontiguousarray(
                    Wh.reshape(128, NCC, 128).transpose(2, 1, 0).reshape(128, C)
                )
        Wv_half = Wv[r0 : r0 + HL * HD]  # [1024, C]
        wv_in = np.empty((2, 128, 16 * SW), np.float32)
        for qd in range(2):
            Wv4 = Wv_half[qd * SW : (qd + 1) * SW]  # [512 d4, C]
            wv_in[qd] = (
                Wv4.reshape(SW, NCC, 128).transpose(2, 1, 0).reshape(128, NCC * SW)
            )
        wp_in = np.ascontiguousarray(
            Wp.T[r0 : r0 + HL * HD].reshape(HL, 128, C)
        )
        halves.append((wq_in, wk_in, wv_in, wp_in))

    in_maps = []
    for b in range(B):
        xTb = np.ascontiguousarray(x[b].T).reshape(NCC, 128, T)
        for half in range(2):
            wq_in, wk_in, wv_in, wp_in = halves[half]
            in_maps.append(
                {
                    "xT": xTb,
                    "wq": wq_in,
                    "wk": wk_in,
                    "wv": wv_in,
                    "wp": wp_in,
                    "cs": cs,
                    "ss": ss,
                    "ones_in": np.ones((128, 128), np.float32),
                }
            )
    return in_maps


_RUNNER_CACHE = None


class _Runner:
    def __init__(self, sharded, mesh, in_names, out_names, out_avals, zero_shapes):
        self.sharded = sharded
        self.mesh = mesh
        self.in_names = in_names
        self.out_names = out_names
        self.out_avals = out_avals
        self.zero_shapes = zero_shapes

    def concat_inputs(self, in_maps):
        return [
            np.concatenate([np.asarray(m[name]) for m in in_maps], axis=0)
            for name in self.in_names
        ]

    def make_zeros(self):
        return [np.zeros((N_CORES * s[0], *s[1:]), d) for (s, d) in self.zero_shapes]

    def run(self, in_maps):
        out_arrs = self.sharded(*self.concat_inputs(in_maps), *self.make_zeros())
        return [
            {
                name: np.asarray(out_arrs[i]).reshape(
                    N_CORES, *self.out_avals[i].shape
                )[c]
                for i, name in enumerate(self.out_names)
            }
            for c in range(N_CORES)
        ]


def _make_runner(nc=None):
    """Compile the Bass program once and return a _Runner that reuses the
    jitted executable across calls. Mirrors bass2jax.run_bass_via_pjrt's
    multi-core branch."""
    import jax
    from jax.experimental.shard_map import shard_map
    from jax.sharding import Mesh, PartitionSpec

    if nc is None:
        nc = build_program()
    bass2jax.install_neuronx_cc_hook()

    partition_name = nc.partition_id_tensor.name if nc.partition_id_tensor else None
    in_names, out_names, out_avals, zero_shapes = [], [], [], []
    for alloc in nc.m.functions[0].allocations:
        if not isinstance(alloc, mybir.MemoryLocationSet):
            continue
        name = alloc.memorylocations[0].name
        if alloc.kind == "ExternalInput":
            if name != partition_name:
                in_names.append(name)
        elif alloc.kind == "ExternalOutput":
            shape = tuple(alloc.tensor_shape)
            dtype = mybir.dt.np(alloc.dtype)
            out_names.append(name)
            out_avals.append(jax.core.ShapedArray(shape, dtype))
            zero_shapes.append((shape, dtype))
    n_params = len(in_names)
    n_outs = len(out_avals)
    all_in_names = list(in_names) + list(out_names)
    if partition_name is not None:
        all_in_names.append(partition_name)
    donate = tuple(range(n_params, n_params + n_outs))

    def _body(*args):
        operands = list(args)
        if partition_name is not None:
            operands.append(bass2jax.partition_id_tensor())
        outs = bass2jax._bass_exec_p.bind(
            *operands,
            out_avals=tuple(out_avals),
            in_names=tuple(all_in_names),
            out_names=tuple(out_names),
            lowering_input_output_aliases=(),
            sim_require_finite=True,
            sim_require_nnan=True,
            nc=nc,
        )
        return tuple(outs)

    devices = jax.devices()[:N_CORES]
    mesh = Mesh(np.asarray(devices), ("core",))
    in_specs = (PartitionSpec("core"),) * (n_params + n_outs)
    out_specs = (PartitionSpec("core"),) * n_outs
    sharded = jax.jit(
        shard_map(
            _body, mesh=mesh, in_specs=in_specs, out_specs=out_specs, check_rep=False
        ),
        donate_argnums=donate,
        keep_unused=True,
    )
    return _Runner(sharded, mesh, in_names, out_names, out_avals, zero_shapes)


def get_runner():
    global _RUNNER_CACHE
    if _RUNNER_CACHE is None:
        _RUNNER_CACHE = _make_runner()
    return _RUNNER_CACHE


def kernel(x, Wq, Wk, Wv, Wp):
    runner = get_runner()
    in_maps = prepare_core_inputs(
        np.asarray(x), np.asarray(Wq), np.asarray(Wk), np.asarray(Wv), np.asarray(Wp)
    )
    res = runner.run(in_maps)
    out = np.empty((B, T, C), np.float32)
    for b in range(B):
        np.add(res[2 * b]["out"], res[2 * b + 1]["out"], out=out[b])
    return out
